# revision 12
# baseline (speedup 1.0000x reference)
"""Trainium2 Bass kernel for nn_DecoderBlock (self-attn + cross-attn + MLP), 8 cores.

Sharding: data-parallel over (batch, sequence-half): core c handles batch b=c//2
and query rows [h*1024,(h+1)*1024); host permutes x rows so the core's query
rows are rows 0:1024.  K/V computed redundantly per pair (no collectives).

Key structure vs the original baseline:
  - Softmax denominator folded into AV: ones column appended to the V
    stationary, AV runs as fp8 DoubleRow (stationary [128,2,80] = 2 key-chunk
    k-tiles x (64 dims + ones + pad), moving ats [128,2,512] fp8).
  - Q/K/ats in fp8e4: scores at bf16 rate, AV at DoubleRow rate.
  - Weights host-pretiled to [128, K/128, O]: contiguous weight DMAs.
  - LN transposes fused: one 3D dma_start_transpose per [128,768] tile.
  - Software pipelining: independent GEMM chunks (cross K/V/Q, o-proj, fc1)
    are pumped into the PE stream between attention iterations so the PE
    stays busy while ACT streams the exps.
  - gelu batched after all attention exps (ACT table thrash avoidance).
  - SBUF pools managed as two LIFO stacks (left/right sides) by lifetime era.
"""

import contextlib

import numpy as np
import ml_dtypes

import concourse.bass as bass
import concourse.mybir as mybir
import concourse.tile as tile
from concourse import bacc
from concourse.bass import ds, ts
from concourse.bass_utils import run_bass_kernel_spmd

FP32 = mybir.dt.float32
BF16 = mybir.dt.bfloat16
FP8 = mybir.dt.float8e4
AF = mybir.ActivationFunctionType
ALU = mybir.AluOpType
DR = mybir.MatmulPerfMode.DoubleRow

B, N, C, H = 4, 2048, 768, 12
D = C // H            # 64
HID = 4 * C           # 3072
NQ = N // 2           # 1024 queries per core
EPS = 1e-5
SCALE = float(D) ** -0.5
NCH = C // 128        # 6
NT_ALL = N // 128     # 16
NT_Q = NQ // 128      # 8
NHP = H // 2          # 6
WS = 256.0            # host-side weight scale (fp8 denormal avoidance)
RWS = 1.0 / WS


class _Prog:
    pass


def _build(P):
    nc = P.nc
    tc = P.tc
    ctx = P.ctx

    # --------- persistent pools (left stack bottom) ---------
    consts = ctx.enter_context(tc.tile_pool(name="consts", bufs=1))
    small = ctx.enter_context(tc.tile_pool(name="small", bufs=2))
    xresp = ctx.enter_context(tc.tile_pool(name="xres", bufs=NT_Q))
    lnbig = ctx.enter_context(tc.tile_pool(name="lnbig", bufs=1))
    gemmps = ctx.enter_context(tc.tile_pool(name="gemmps", bufs=1, space="PSUM"))
    # xov: xn staging (x2), attention output, V-with-ones; era1 -> post-cross
    xov = tc.alloc_tile_pool(name="xov", bufs=1)
    # selfp: self-era weights, y staging, q/k, ln2; era1 -> post-self
    selfp = tc.alloc_tile_pool(name="selfp", bufs=1)

    ones_row = consts.tile([1, 128], BF16, tag="ones_row", name="ones_row")
    nc.vector.memset(ones_row[:], 1.0)
    if P.with_bias:
        qkb = consts.tile([128, 12], FP32, tag="qkb", name="qkb")
        nc.sync.dma_start(qkb[:], P.qkb_d.rearrange("(j p) -> p j", p=128))
        qkb2 = consts.tile([128, 12], FP32, tag="qkb2", name="qkb2")
        nc.sync.dma_start(qkb2[:], P.qkb2_d.rearrange("(j p) -> p j", p=128))
        fc1b = consts.tile([128, 24], FP32, tag="fc1b", name="fc1b")
        nc.sync.dma_start(fc1b[:], P.fc1b_d.rearrange("(j p) -> p j", p=128))
        brows = []
        for i in range(5):
            r = consts.tile([1, C], BF16, tag=f"brow{i}", name=f"brow{i}")
            nc.sync.dma_start(r[:], P.brows_d[i:i + 1, :])
            brows.append(r)
        brow_v_sa, brow_o_sa, brow_v_ca, brow_o_ca, brow_fc2 = brows
    else:
        qkb = qkb2 = fc1b = None
        brow_v_sa = brow_o_sa = brow_v_ca = brow_o_ca = brow_fc2 = None

    # ---------- helpers ----------
    def rsqrt_dve(var_ap, rstd):
        v = small.tile([128, 1], FP32, tag="rs_v", name="rs_v")
        nc.vector.tensor_scalar_add(v[:], var_ap, EPS)
        yi = small.tile([128, 1], mybir.dt.int32, tag="rs_yi", name="rs_yi")
        nc.vector.tensor_scalar(yi[:], v[:].bitcast(mybir.dt.int32), 1, -1,
                                ALU.arith_shift_right, ALU.bitwise_xor)
        y = small.tile([128, 1], FP32, tag="rs_y", name="rs_y")
        nc.vector.tensor_scalar_add(y[:].bitcast(mybir.dt.int32), yi[:],
                                    0x5F3759E0)
        t1 = small.tile([128, 1], FP32, tag="rs_t1", name="rs_t1")
        t2 = small.tile([128, 1], FP32, tag="rs_t2", name="rs_t2")
        for _ in range(2):
            nc.vector.tensor_tensor(t1[:], y[:], y[:], ALU.mult)
            nc.vector.tensor_tensor(t2[:], t1[:], v[:], ALU.mult)
            nc.vector.tensor_scalar(t1[:], t2[:], -0.5, 1.5, ALU.mult, ALU.add)
            nc.vector.tensor_tensor(rstd[:], y[:], t1[:], ALU.mult)
            y, rstd = rstd, y
        return y

    P.xn_i = 0

    def ln_tile(xt, lnt_slice, conv=None):
        """LN one [128,768] fp32 tile -> bf16 -> 3D transpose -> fp8 lnt
        slice [128, 6, 128]."""
        st = small.tile([128, 2, 6], FP32, tag="ln_st", name="ln_st")
        nc.vector.bn_stats(st[:, 0, :], xt[:, 0:384])
        nc.vector.bn_stats(st[:, 1, :], xt[:, 384:768])
        mv = small.tile([128, 2], FP32, tag="ln_mv", name="ln_mv")
        nc.vector.bn_aggr(mv[:], st[:])
        rstd0 = small.tile([128, 1], FP32, tag="ln_rstd", name="ln_rstd")
        rstd = rsqrt_dve(mv[:, 1:2], rstd0)
        xn = xov.tile([128, 768], BF16, tag=f"xn{P.xn_i % 2}", name="ln_xn")
        tst = xov.tile([128, NCH, 128], BF16, tag=f"tst{P.xn_i % 2}",
                       name="tst")
        P.xn_i += 1
        nc.vector.tensor_scalar(xn[:], xt[:], mv[:, 0:1], rstd[:],
                                ALU.subtract, ALU.mult)
        nc.sync.dma_start_transpose(tst[:], xn[:])
        (conv or nc.vector).tensor_copy(lnt_slice, tst[:])

    def qk_chunk(wsb, col_j, lnt, tok0, ntok, out_tt, out_j, bias_tile,
                 bias_off, dst_tok0=None):
        """One j-chunk of a weight-stationary GEMM:
        out_tt[:, out_j, dst_tok0:+ntok] = (W 128-col-chunk).T @ LN^T (+b)."""
        if dst_tok0 is None:
            dst_tok0 = tok0
        ps = P.cur_ps.tile([128, 1024], FP32, tag="gps", name="gps")
        for c2 in range(NCH // 2):
            for half in range(ntok // 512):
                nc.tensor.matmul(
                    ps[:, ds(half * 512, 512)],
                    wsb[:, ds(2 * c2, 2), ts(col_j, 128)],
                    lnt[:, ds(2 * c2, 2), ds(tok0 + half * 512, 512)],
                    start=(c2 == 0), stop=(c2 == NCH // 2 - 1), perf_mode=DR)
        dst = out_tt[:, out_j, ds(dst_tok0, ntok)]
        src = ps[:, 0:ntok]
        bias = (bias_tile[:, bias_off + out_j:bias_off + out_j + 1]
                if bias_tile is not None else 0.0)
        nc.any.tensor_scalar(dst, src, RWS, bias, ALU.mult, ALU.add)

    def nat_chunk(src_tt, ki_list, wsb, wk0, bias_row, t, consumer,
                  psum_acc=None, first=True, last=True, dr=True):
        """One 128-token chunk of a natural GEMM:
        ps[128,768] (+)= sum_ki src_tt[:,ki,t*128:].T @ wsb[:,wk0+n,:] (+bias)."""
        ps = psum_acc if psum_acc is not None else P.cur_ps.tile(
            [128, 1024], FP32, tag="gps", name="gps")
        npair = len(ki_list) // 2
        for sl in (slice(0, 512), slice(512, 768)):
            if dr:
                for n2 in range(npair):
                    kp = ki_list[2 * n2]
                    nc.tensor.matmul(
                        ps[:, sl], src_tt[:, ds(kp, 2), ts(t, 128)],
                        wsb[:, ds(wk0 + 2 * n2, 2), sl],
                        start=(first and n2 == 0),
                        stop=(last and bias_row is None
                              and n2 == npair - 1), perf_mode=DR)
            else:
                for n, ki in enumerate(ki_list):
                    nc.tensor.matmul(ps[:, sl], src_tt[:, ki, ts(t, 128)],
                                     wsb[:, wk0 + n, sl],
                                     start=(first and n == 0),
                                     stop=(last and bias_row is None
                                           and n == len(ki_list) - 1))
            if last and bias_row is not None:
                nc.tensor.matmul(ps[:, sl], ones_row[0:1, :], bias_row[:, sl],
                                 start=False, stop=True)
        if last:
            consumer(t, ps)
        return ps

    # ---------- pump machinery ----------
    queue = []

    def pump(budget_us):
        while queue and budget_us > 0:
            est, fn = queue.pop(0)
            fn()
            budget_us -= est

    # ---------- one attention iteration ----------
    P.at_i = 0
    P.pending_norm = None

    def flush_norm():
        """Emit the deferred normalize of the previous attention iteration.
        Deferring it past the next iteration's first scores keeps the PE from
        stalling on the recip->broadcast->mult chain at iteration boundaries."""
        if P.pending_norm is None:
            return
        pos, ot, hp, qsl = P.pending_norm
        P.pending_norm = None
        rbp = P.scps.tile([128, 2, 512], FP32, tag="sc", name="rbp")
        for hh in range(2):
            rc = small.tile([1, 512], BF16, tag="rc", name="rc")
            with nc.allow_low_precision(reason="softmax denom recip"):
                nc.vector.reciprocal(rc[:], pos[hh][64:65, :])
            nc.tensor.matmul(rbp[0:64, hh, :], ones_row[0:1, 0:64], rc[:],
                             start=True, stop=True)
            # DVE reads at most one PSUM operand: stage po rows via ACT copy
            osb = small.tile([64, 512], FP32, tag="osb", name="osb")
            nc.scalar.copy(osb[:], pos[hh][0:64, :])
            nc.vector.tensor_tensor(ot[ds(hh * 64, 64), hp, qsl],
                                    osb[:], rbp[0:64, hh, :], ALU.mult)

    def attention_iter(tq, hp, qt, kt, vv, ot, per_e=0.0, post=9.0):
        qsl = ts(tq, 512)
        pos = [None, None]
        ats = {}

        def scores(e):
            for hh in range(2):
                sc = P.scps.tile([128, 2, 512], FP32, tag="sc", name="sc")
                for i in range(2):
                    kc = e * 2 + i
                    nc.tensor.matmul(
                        sc[:, i, :], kt[ds(hh * 64, 64), hp, ts(kc, 128)],
                        qt[ds(hh * 64, 64), hp, qsl], start=True, stop=True,
                        tile_position=(hh * 64, 0))
                if (P.at_i * 2) % 5 >= 2:
                    a = P.crossp.tile([128, 2, 512], FP8,
                                      tag=f"at{P.at_i % 6}", name="at")
                    nc.scalar.activation(a[:], sc[:], AF.Exp, scale=SCALE)
                    ats[(hh, e)] = a[:]
                else:
                    # DVE Schraudolph: fp8e4 bits ~= 8*log2(exp(s*SCALE)) + 56
                    a = P.crossp.tile([128, 2, 512], mybir.dt.int8,
                                      tag=f"at{P.at_i % 6}", name="at")
                    nc.vector.tensor_scalar(
                        a[:], sc[:], 8 * 1.4426950408889634 * SCALE, 56.0,
                        ALU.mult, ALU.add)
                    ats[(hh, e)] = a[:].bitcast(FP8)
                P.at_i += 1

        def av(e):
            for hh in range(2):
                nc.tensor.matmul(
                    pos[hh][:], vv[:, e, 2 * hp + hh, :, :], ats[(hh, e)],
                    start=(e == 0), stop=(e == 7), perf_mode=DR)

        for e in range(8):
            scores(e)
            if e == 0:
                flush_norm()
                pos[0] = P.avpo.tile([80, 512], FP32, tag="po", name="po")
                pos[1] = P.avpo.tile([80, 512], FP32, tag="po", name="po")
            if per_e > 0:
                pump(per_e)
            if e > 0:
                av(e - 1)
        av(7)
        P.pending_norm = (pos, ot, hp, qsl)
        if post > 0:
            pump(post)

    # ================= emission =================

    # ---------- era 1: loads, LN1, self QKV, LN(y) ----------
    xres = [xresp.tile([128, 768], FP32, tag="xres", name="xres")
            for _ in range(NT_Q)]

    with tc.tile_pool(name="xtmp", bufs=4) as xtmpp, \
            tc.tile_pool(name="wqkvp", bufs=1) as wqkvp, \
            tc.tile_pool(name="qkvps", bufs=2, space="PSUM") as qkvps:
        P.cur_ps = qkvps
        nc.sync.dma_start(xres[0][:], P.x_d[ts(0, 128), :])
        # wqkv: V-columns first so the V gemm (which only needs one LN tile
        # per chunk) can start as early as possible.
        wqkv_sb = wqkvp.tile([128, NCH, 3 * C], FP8, tag="wqkv", name="wqkv")
        nc.sync.dma_start(wqkv_sb[:, :, 2 * C:3 * C], P.wqkv_d[:, :, 2 * C:3 * C])
        for t in range(1, NT_Q):
            nc.sync.dma_start(xres[t][:], P.x_d[ts(t, 128), :])
        nc.sync.dma_start(wqkv_sb[:, :, 0:2 * C], P.wqkv_d[:, :, 0:2 * C])
        wk_sb = selfp.tile([128, NCH, C], FP8, tag="wk", name="wk_sb")
        nc.sync.dma_start(wk_sb[:], P.wk_d)
        wosa_sb = selfp.tile([128, NCH, C], FP8, tag="wosa", name="wosa_sb")
        nc.sync.dma_start(wosa_sb[:], P.wo_sa_d)
        wq_sb = selfp.tile([128, NCH, C], FP8, tag="wq", name="wq_sb")
        nc.sync.dma_start(wq_sb[:], P.wq_d)

        qt = selfp.tile([128, NHP, NQ], FP8, tag="qt", name="qt")
        kt = selfp.tile([128, NHP, N], FP8, tag="kt", name="kt")
        vv = xov.tile([128, 8, H, 2, 80], FP8, tag="vv", name="vv")
        ot = xov.tile([128, NHP, NQ], FP8, tag="ot", name="ot")

        def fill_v_chunk(vv_t, wsb, lnt, brow, t, eng=None):
            def vcons(t, ps):
                (eng or nc.any).tensor_scalar(
                    vv_t[:, t // 2, :, t % 2, 0:64],
                    ps[:, 0:768].rearrange("p (h d) -> p h d", h=H),
                    RWS, 0.0, ALU.mult, ALU.add)
            nat_chunk(lnt, list(range(NCH)), wsb, 0, brow, t, vcons)

        nc.vector.memset(vv[:, :, :, :, 64:80], 0.0)
        nc.vector.memset(vv[:, :, :, :, 64:65], 1.0)

        # LN1 (x) interleaved with V chunks (V chunk t needs only LN tile t)
        ln1t = lnbig.tile([128, NCH, N], FP8, tag="lnbig", name="ln1t")
        wqkv_v = wqkv_sb[:, :, 2 * C:3 * C]
        for t in range(NT_Q):
            ln_tile(xres[t], ln1t[:, 0:NCH, ts(t, 128)], conv=nc.any)
            fill_v_chunk(vv, wqkv_v, ln1t, brow_v_sa, t)
        for t in range(NT_Q, NT_ALL):
            xt = xtmpp.tile([128, 768], FP32, tag="xtmp", name="xtmp")
            nc.sync.dma_start(xt[:], P.x_d[ts(t, 128), :])
            ln_tile(xt, ln1t[:, 0:NCH, ts(t, 128)], conv=nc.any)
            fill_v_chunk(vv, wqkv_v, ln1t, brow_v_sa, t)

        # self Q then K
        for j in range(NHP):
            qk_chunk(wqkv_sb, j, ln1t, 0, 1024, qt, j, qkb, 0)
        for j in range(NHP):
            qk_chunk(wqkv_sb, NHP + j, ln1t, 0, 1024, kt, j, qkb, 6)
            qk_chunk(wqkv_sb, NHP + j, ln1t, 1024, 1024, kt, j, qkb, 6)

        # y loads on the ACT hwdge queue + LN(y) -> lnyt (reuses ln1t's slot,
        # so y-LN starts once self-QKV has finished reading ln1t).
        lnyt = lnbig.tile([128, NCH, N], FP8, tag="lnbig", name="lnyt")
        for t in range(NT_ALL):
            yt = selfp.tile([128, 768], FP32, tag=f"yst{t % 2}", name="yst")
            nc.scalar.dma_start(yt[:], P.y_d[ts(t, 128), :])
            ln_tile(yt, lnyt[:, 0:NCH, ts(t, 128)], conv=nc.any)

    P.cur_ps = gemmps

    # ---------- era 2: self attention + pumped crossK/o_sa/ln2/crossQ ----------
    # right-stack pool: cross q/k + ats (lives to end)
    P.crossp = tc.alloc_tile_pool(name="crossp", bufs=1, side="right")
    P.scps = tc.alloc_tile_pool(name="scps", bufs=2, space="PSUM")
    P.avpo = tc.alloc_tile_pool(name="avpo", bufs=2, space="PSUM")

    ln2t = selfp.tile([128, NCH, NQ], FP8, tag="ln2t", name="ln2t")
    qt2 = P.crossp.tile([128, NHP, NQ], FP8, tag="qt2", name="qt2")
    kt2 = P.crossp.tile([128, NHP, N], FP8, tag="kt2", name="kt2")
    wv_sb = P.crossp.tile([128, NCH, C], FP8, tag="wv", name="wv_sb")
    nc.sync.dma_start(wv_sb[:], P.wv_d)
    vv2 = xov.tile([128, 8, H, 2, 80], FP8, tag="vv2", name="vv2")
    nc.vector.memset(vv2[:, :, :, :, 64:80], 0.0)
    nc.vector.memset(vv2[:, :, :, :, 64:65], 1.0)

    for j in range(NHP):
        def ck(j=j):
            qk_chunk(wk_sb, j, lnyt, 0, 1024, kt2, j, qkb2, 6)
            qk_chunk(wk_sb, j, lnyt, 1024, 1024, kt2, j, qkb2, 6)
        queue.append((5.2, ck))
    for t in range(NT_ALL):
        queue.append((1.4, lambda t=t: fill_v_chunk(
            vv2, wv_sb, lnyt, brow_v_ca, t, eng=nc.any)))

    def res_cons(t, ps):
        osc = small.tile([128, 768], FP32, tag="osc", name="osc")
        nc.scalar.activation(osc[:], ps[:, 0:768], AF.Copy, scale=RWS)
        nc.vector.tensor_tensor(xres[t][:], osc[:], xres[t][:], ALU.add)

    def osa_chunk(t):
        nat_chunk(ot, list(range(NHP)), wosa_sb, 0, brow_o_sa, t, res_cons)

    for tq in range(2):
        for hp in range(NHP):
            attention_iter(tq, hp, qt, kt, vv, ot)
        for t in range(tq * 4, tq * 4 + 4):
            queue.append((2.3, lambda t=t: osa_chunk(t)))
            queue.append((0.3, lambda t=t: ln_tile(
                xres[t], ln2t[:, 0:NCH, ts(t, 128)])))
        for j in range(NHP):
            queue.append((1.4, lambda j=j, tq=tq: qk_chunk(
                wq_sb, j, ln2t, tq * 512, 512, qt2, j, qkb2, 0)))
    # NOTE: the tq1 leftovers (o_sa, ln2, crossQ) stay queued; they drain
    # during cross-attn tq0 so cross scores/exps start immediately.

    # ---------- era 3: cross attention + pumped crossV/o_ca/ln3/fc1(tq0) ----
    w9b = tc.alloc_tile_pool(name="w9b", bufs=1, side="right")
    woca_sb = w9b.tile([128, NCH, C], FP8, tag="woca", name="woca_sb")
    nc.sync.dma_start(woca_sb[:], P.wo_ca_d)

    # (vv2/crossV enqueued during the self-attn era; see above)

    ot2 = xov.tile([128, NHP, NQ], FP8, tag="ot", name="ot2")
    ln3t = lnbig.tile([128, NCH, NQ], FP8, tag="lnbig", name="ln3t")

    def oca_chunk(t):
        nat_chunk(ot2, list(range(NHP)), woca_sb, 0, brow_o_ca, t, res_cons)

    w1h = [None, None]
    hpre = [None]

    def fc1_chunk(h, jloc, tq):
        jglob = h * 12 + jloc
        qk_chunk(w1h[h], jloc, ln3t, tq * 512, 512, hpre[0], jglob, fc1b, 0,
                 dst_tok0=0)

    hwsp = None
    for tq in range(2):
        for hp in range(NHP):
            per_e = 3.0 if (tq == 0 and hp <= 1) else 0.0
            attention_iter(tq, hp, qt2, kt2, vv2, ot2, per_e=per_e)
            if tq == 0 and hp == 2:
                # self-era leftovers and crossV are fully drained by now;
                # release selfp before hwsp pushes (SBUF headroom).
                pump(1e9)
                selfp.release()
        for t in range(tq * 4, tq * 4 + 4):
            queue.append((2.3, lambda t=t: oca_chunk(t)))
            queue.append((0.3, lambda t=t: ln_tile(
                xres[t], ln3t[:, 0:NCH, ts(t, 128)])))
        if tq == 0:
            # right-stack pool: hpre (per-tq) + weight halves (w1 then w2)
            hwsp = tc.alloc_tile_pool(name="hwsp", bufs=1, side="right")
            for h in range(2):
                w1h[h] = hwsp.tile([128, NCH, HID // 2], FP8, tag=f"ws{h}",
                                   name=f"w1h{h}")
                nc.sync.dma_start(w1h[h][:], P.w1_d[:, :, ds(h * 1536, 1536)])
            hpre[0] = hwsp.tile([128, 24, 512], BF16, tag="hp", name="hpre0")
            for h in range(2):
                for jloc in range(12):
                    queue.append((1.4, lambda h=h, j=jloc: fc1_chunk(h, j, 0)))

    # post-cross: drain (o_ca tq1, ln3 tq1, fc1 tq0 leftovers) on the tail psum
    P.avpo.release()
    P.scps.release()
    tailps = tc.alloc_tile_pool(name="tailps", bufs=2, space="PSUM")
    P.cur_ps = tailps
    pump(1e9)

    xov.release()

    # ---------- era 4: gelu(tq0); fc1(tq1); w2; fc2(tq0); gelu+fc2(tq1) ----
    with tc.tile_pool(name="htp", bufs=1) as htp, \
            tc.tile_pool(name="ostp", bufs=2) as ostp:
        w2h = [None, None]

        def gelu_tq(dst_ht, src_hpre):
            for t in range(4):
                nc.scalar.activation(dst_ht[:, :, ts(t, 128)],
                                     src_hpre[:, :, ts(t, 128)], AF.Gelu)

        def fc2_tq(tq, ht):
            for t in range(4):
                tg = tq * 4 + t
                ps = nat_chunk(ht, list(range(12)), w2h[0], 0, None, t, None,
                               first=True, last=False, dr=False)

                def cons(_t, ps, tg=tg):
                    ost = ostp.tile([128, 768], FP32, tag="ost", name="ost")
                    nc.vector.tensor_tensor(ost[:], ps[:, 0:768], xres[tg][:],
                                            ALU.add)
                    nc.sync.dma_start(P.out_d[ts(tg, 128), :], ost[:])
                nat_chunk(ht, list(range(12, 24)), w2h[1], 0, brow_fc2, t,
                          cons, psum_acc=ps, first=False, last=True, dr=False)

        ht0 = htp.tile([128, 24, 512], BF16, tag="ht", name="ht0")
        gelu_tq(ht0, hpre[0])
        hpre[0] = hwsp.tile([128, 24, 512], BF16, tag="hp", name="hpre1")
        for h in range(2):
            for jloc in range(12):
                fc1_chunk(h, jloc, 1)
        for h in range(2):
            w2h[h] = hwsp.tile([128, 12, C], BF16, tag=f"ws{h}", name=f"w2h{h}")
            nc.sync.dma_start(w2h[h][:], P.w2_d[:, ds(h * 12, 12), :])
        fc2_tq(0, ht0)
        ht1 = htp.tile([128, 24, 512], BF16, tag="ht", name="ht1")
        gelu_tq(ht1, hpre[0])
        fc2_tq(1, ht1)

    hwsp.release()
    w9b.release()
    P.crossp.release()
    tailps.release()
